# revision 37
# baseline (speedup 1.0000x reference)
"""DHN pairwise-loss kernel for Trainium2 (Bass/Tile), 8-core SPMD.

Math (reference, per row i of sim = 0.5*b@b.T, pos = same-label mask):
    t[p,n]   = theta[p] - theta[n] - ALPHA          (fp32 clip is a no-op here)
    val[p,n] = log1p(exp(t)) - t = ln(v_n + uinv_p) + ln(u_p),
               with u_p = e^{theta_p - ALPHA}, v_n = e^{-theta_n}
    row_loss = sum over (p in pos, n in ~pos) val / (n_pos*n_neg)
    loss1    = mean(row_loss); loss2 = mean((b - sign(b))^2); total = loss1 + loss2

Device mapping v2 (per core: 256 rows as 2 chunks of 128 partitions):
    SLOT-PAIRING: for a pair of positive slots (a, b) of the same row,
        ln(v + u_a) + ln(v + u_b) = ln(v^2 + v*(u_a+u_b) + u_a*u_b)
    so with per-chunk tiles v = Exp(-sim') (bf16) and Q = v*v (bf16), each
    slot-PAIR m needs only two native bf16 DVE ops over [128, N]:
        t1 = v * sigma_m + pi_m      (tensor_scalar, 4x bf16 mode)
        t  = t1 + Q                  (tensor_tensor, 2x bf16 mode)
    and the ACT engine runs ONE batched Ln over G pairs' t tiles with a
    single accumulator read -- no per-slot bias, no per-slot accum read.
    sim' = 0.5*b@b.T + MASKC*Y@Y.T pushes same-label pairs to theta+100 so
    v underflows to ~0 there and t == pi exactly; pad slots use u = C_PAD/u_a
    so masked-pad products pi = C_PAD stay inside Ln's accurate input range
    [~2.5e-19, 2^64].  Host folds all analytic terms (sum of t over real
    pairs, masked/pad contributions) into one per-row fp64 constant C;
    device computes (sum_ln + C) * w per row and PE-sums over partitions.

Per-row pairing (host): sort the row's u values descending, pair u[m] with
u[2M-1-m]; rows with fewer positives than slots pair their largest u's with
pads C_PAD/u.  Requires count >= M for every row of a chunk (guarded, with
the v1 column-pairing kernel kept as fallback).
"""

import os
import numpy as np

N = 2048
D = 64
ALPHA = 5.0
LAMBDA = 1.0
NCORES = 8
MASKC = 100.0  # same-label sim offset: v = e^-(theta+100) ~ 0
LN_CPAD = -40.0  # pad pair product pi = e^-40, inside Ln accurate range
GSIZE = 12     # slot-pairs per batched Ln instruction

# v1 fallback constants
PAD_A = 43.0
C_PAD = 43.0

LAST_RESULTS = None  # BassKernelResults of the most recent run (for test harness)

_CACHE = {}


def _row_stats(b, y):
    b64 = np.asarray(b, dtype=np.float64)
    labels, inv, counts = np.unique(y, return_inverse=True, return_counts=True)
    ncls = len(labels)
    n_row = counts[inv]
    cls_idx = [np.nonzero(inv == c)[0] for c in range(ncls)]
    pos_theta = [None] * len(y)
    for ix in cls_idx:
        g = 0.5 * (b64[ix] @ b64[ix].T)
        for k, r in enumerate(ix):
            pos_theta[r] = g[k]
    s_allth = 0.5 * (b64 @ b64.sum(axis=0))
    s_posth = np.array([pt.sum() for pt in pos_theta])
    return inv, ncls, n_row, pos_theta, s_allth, s_posth


def _host_prep_v2(b, y):
    """Slot-pairing prep: per-core brt + sigma/pi/tw tiles, shared bth."""
    b = np.ascontiguousarray(np.asarray(b, dtype=np.float32))
    y = np.asarray(y, dtype=np.int64).ravel()
    n = b.shape[0]
    assert b.shape == (N, D) and y.shape == (N,), (b.shape, y.shape)

    inv, ncls, n_row, pos_theta, s_allth, s_posth = _row_stats(b, y)
    order = np.argsort(-n_row, kind="stable")
    slot_rows = [order[: n // 2], order[n // 2:]]
    P0 = int(n_row[slot_rows[0]].max())
    P1 = int(n_row[slot_rows[1]].max())
    Ms = [(P0 + 1) // 2, (P1 + 1) // 2]

    # precondition: every row has at least one real u per pair
    if int(n_row[slot_rows[0]].min()) < Ms[0] or \
       int(n_row[slot_rows[1]].min()) < Ms[1]:
        return None

    # pairs migrated to the ACT-biased path (engine balance): must be
    # real-real for every chunk0 row
    nmig = int(os.environ.get("BASS_DHN_NMIG", "2"))
    npad_max0 = 2 * Ms[0] - int(n_row[slot_rows[0]].min())
    if Ms[0] - nmig < npad_max0:
        nmig = 0

    valid = (n_row >= 1) & (n_row < n)
    cnt = int(valid.sum())
    npairs = n_row.astype(np.float64) * (n - n_row)
    w_all = np.where(valid, 1.0 / np.maximum(npairs, 1.0) / max(cnt, 1), 0.0)

    # range guard: worst pair product (v_max + a)(v_max + b) must stay below
    # Ln's accurate ceiling 2^64; v_max per row over non-class columns.
    sim_h = 0.5 * (b @ b.T).astype(np.float64)
    offmask = np.where(y[:, None] == y[None, :], 1e9, sim_h)
    vmax = np.exp(-offmask.min(axis=1))

    cpad = float(np.exp(LN_CPAD))
    onehot = np.zeros((n, ncls), dtype=np.float32)
    onehot[np.arange(n), inv] = 1.0
    import ml_dtypes
    bf = ml_dtypes.bfloat16
    bth = np.concatenate([0.5 * b.T, onehot.T], axis=0)
    bth = np.ascontiguousarray(bth.astype(bf))               # [D+C, N] shared

    MT = Ms[0] + Ms[1]
    in_maps = []
    worst_t = 0.0
    min_pi = np.inf
    for core in range(NCORES):
        chunks = [slot_rows[0][core * 128:(core + 1) * 128],
                  slot_rows[1][core * 128:(core + 1) * 128]]
        rows = np.concatenate(chunks)
        brt = np.concatenate([b[rows].T, MASKC * onehot[rows].T], axis=0)
        brt = np.ascontiguousarray(brt.astype(bf))           # [D+C, 256]
        sig = np.zeros((128, MT), dtype=np.float32)
        pi = np.zeros((128, MT), dtype=np.float32)
        tw = np.zeros((128, 4), dtype=np.float32)
        for s, chunk in enumerate(chunks):
            M = Ms[s]
            off = 0 if s == 0 else Ms[0]
            for k, r in enumerate(chunk):
                uu = np.sort(np.exp(ALPHA - pos_theta[r]))[::-1]
                ncr = uu.size
                npad = 2 * M - ncr
                a = uu[:M]
                j = 2 * M - 1 - np.arange(M)
                real = j <= ncr - 1
                bb = np.where(real, uu[np.minimum(j, ncr - 1)], cpad / a)
                pim = np.where(real, a * bb, cpad)
                sigk = a + bb
                pik = pim.copy()
                if s == 0 and nmig:
                    # migrated pairs carry raw (a, b) as Ln biases instead
                    sigk[M - nmig:] = a[M - nmig:]
                    pik[M - nmig:] = bb[M - nmig:]
                    min_pi = min(min_pi, float(a[M - nmig:].min()),
                                 float(bb[M - nmig:].min()))
                sig[k, off:off + M] = sigk
                pi[k, off:off + M] = pik
                G = float((LN_CPAD - np.log(a[~real])).sum())
                C = (2 * M * s_allth[r] - npad * s_posth[r]
                     - ncr * ncr * ALPHA - ncr * G)
                tw[k, 2 * s] = C
                tw[k, 2 * s + 1] = w_all[r]
                wt = float(((vmax[r] + a) * (vmax[r] + bb)).max())
                worst_t = max(worst_t, wt)
                min_pi = min(min_pi, float(pim[:M - nmig if s == 0 else M]
                                           .min()))
        in_maps.append({"brt": brt, "bth": bth, "sig": sig, "pi": pi,
                        "tw": tw})
    if worst_t > 1.0e19 or min_pi < 1.0e-18:
        return None
    return in_maps, Ms[0], Ms[1], ncls, nmig


def _group_sizes(m, g, first=None, last=None):
    """Split m pairs into Ln groups of ~g, optionally with a small first
    group (starts the ACT pipeline early) and small last group (short tail)."""
    sizes = []
    if first is not None and m > first + g:
        sizes.append(first)
        m -= first
    tail = []
    if last is not None and m > last + g:
        tail = [last]
        m -= last
    ng = (m + g - 1) // g
    base = m // ng
    rem = m - base * ng
    sizes += [base + (1 if i < rem else 0) for i in range(ng)]
    return sizes + tail


def _build_bass_v2(Ms0, Ms1, ncls, nmig):
    import concourse.bacc as bacc
    import concourse.tile as tile
    from concourse import mybir

    f32 = mybir.dt.float32
    bf16 = mybir.dt.bfloat16
    AF = mybir.ActivationFunctionType
    OP = mybir.AluOpType
    KD = D + ncls
    MT = Ms0 + Ms1
    ngps = int(os.environ.get("BASS_DHN_GPS", "0"))  # TTs per group on GPSIMD
    bq = int(os.environ.get("BASS_DHN_BQ", "1"))     # broadcast-Q group TT
    inplace = int(os.environ.get("BASS_DHN_INPLACE", "1"))  # Ln writes in tg
    gsizes = [_group_sizes(Ms0 - nmig, GSIZE, first=3),
              _group_sizes(Ms1, GSIZE, last=2)]
    NG = [len(gsizes[0]), len(gsizes[1])]

    nc = bacc.Bacc("TRN2", target_bir_lowering=False, debug=False,
                   num_devices=NCORES)
    brt_d = nc.dram_tensor("brt", [KD, 256], bf16, kind="ExternalInput")
    bth_d = nc.dram_tensor("bth", [KD, N], bf16, kind="ExternalInput")
    sig_d = nc.dram_tensor("sig", [128, MT], f32, kind="ExternalInput")
    pi_d = nc.dram_tensor("pi", [128, MT], f32, kind="ExternalInput")
    tw_d = nc.dram_tensor("tw", [128, 4], f32, kind="ExternalInput")
    out_d = nc.dram_tensor("out", [1, 2], f32, kind="ExternalOutput")

    with tile.TileContext(nc) as tc:
        with (
            tc.tile_pool(name="const", bufs=1) as cpool,
            tc.tile_pool(name="t1p", bufs=3) as t1pool,
            tc.tile_pool(name="tgp", bufs=3) as tgpool,
            tc.tile_pool(name="dmp", bufs=1) as dpool,
            tc.tile_pool(name="small", bufs=2) as mpool,
            tc.tile_pool(name="psum", bufs=2, space="PSUM") as ppool,
            tc.tile_pool(name="psum1", bufs=1, space="PSUM") as ppool1,
        ):
            # brt + first bth slice first so matmul 0 starts ASAP; rest of
            # bth streams in behind it.
            brt = cpool.tile([KD, 256], bf16)
            nc.sync.dma_start(out=brt[:], in_=brt_d[:])
            bth = cpool.tile([KD, N], bf16)
            for q in range(N // 512):
                nc.sync.dma_start(out=bth[:, q * 512:(q + 1) * 512],
                                  in_=bth_d[:, q * 512:(q + 1) * 512])
            sig = cpool.tile([128, MT], f32)
            nc.sync.dma_start(out=sig[:], in_=sig_d[:])
            pit = cpool.tile([128, MT], f32)
            nc.sync.dma_start(out=pit[:], in_=pi_d[:])
            tw = cpool.tile([128, 4], f32)
            nc.sync.dma_start(out=tw[:], in_=tw_d[:])

            ones = cpool.tile([128, 1], f32)
            nc.vector.memset(ones[:], 1.0)

            # loss2 runs first: DVE is otherwise idle while DMA/matmul warm up
            bb = brt[:D, :]
            nb = mpool.tile([D, 256], f32, tag="nb")
            nc.vector.tensor_scalar_mul(nb[:], bb, -1.0)
            ab = mpool.tile([D, 256], f32, tag="ab")
            nc.vector.tensor_max(ab[:], bb, nb[:])
            nc.vector.tensor_scalar_add(ab[:], ab[:], -1.0)
            sq = mpool.tile([D, 256], f32, tag="sq")
            nc.vector.tensor_mul(sq[:], ab[:], ab[:])
            qcol = mpool.tile([D, 1], f32, tag="qcol")
            nc.vector.tensor_reduce(out=qcol[:], in_=sq[:],
                                    axis=mybir.AxisListType.X, op=OP.add)

            # per-chunk v = Exp(-sim') straight out of PE PSUM (1024-wide
            # 2-bank PSUM tiles halve the Exp instruction count), then Q = v*v
            vs, qs = [], []
            for s in range(2):
                v = cpool.tile([128, N], bf16, tag=f"v{s}")
                for q in range(N // 1024):
                    pt = ppool.tile([128, 1024], f32, tag="mm")
                    for h in range(2):
                        cl = slice((2 * q + h) * 512, (2 * q + h + 1) * 512)
                        nc.tensor.matmul(pt[:, h * 512:(h + 1) * 512],
                                         brt[:, s * 128:(s + 1) * 128],
                                         bth[:, cl], start=True, stop=True)
                    nc.scalar.activation(out=v[:, q * 1024:(q + 1) * 1024],
                                         in_=pt[:], func=AF.Exp, scale=-1.0)
                vs.append(v)
                Qt = cpool.tile([128, N], bf16, tag=f"Q{s}")
                qs.append(Qt)
            # DVE order: Q0 immediately, sliced so each piece follows its Exp
            # tile instead of waiting for all of v0; Q1 is emitted inside the
            # chunk0 pair stream so it doesn't stall DVE.
            for q in range(N // 1024):
                sl = slice(q * 1024, (q + 1) * 1024)
                nc.vector.tensor_mul(qs[0][:, sl], vs[0][:, sl],
                                     vs[0][:, sl])

            NL = [NG[0] + 2 * nmig, NG[1]]   # lacc cols per chunk
            lacc = cpool.tile([128, NL[0] + NL[1]], f32, tag="lacc")
            pq = ppool1.tile([1, 1], f32, tag="pq")
            part_sums = []

            def chunk_tail(s, g0):
                la = mpool.tile([128, 1], f32, tag=f"la{s}")
                nc.vector.tensor_reduce(out=la[:],
                                        in_=lacc[:, g0:g0 + NL[s]],
                                        axis=mybir.AxisListType.X,
                                        op=OP.add)
                r3 = mpool.tile([128, 1], f32, tag=f"r3{s}")
                nc.vector.scalar_tensor_tensor(
                    out=r3[:], in0=la[:], scalar=tw[:, 2 * s:2 * s + 1],
                    in1=tw[:, 2 * s + 1:2 * s + 2],
                    op0=OP.add, op1=OP.mult)
                pr = ppool1.tile([1, 1], f32, tag=f"pr{s}")
                nc.tensor.matmul(pr[:], r3[:], ones[:], start=True, stop=True)
                sb = mpool.tile([1, 1], f32, tag=f"sb{s}")
                nc.vector.tensor_copy(out=sb[:], in_=pr[:])
                part_sums.append(sb)

            nc.tensor.matmul(pq[:], qcol[:], ones[:D, :], start=True,
                             stop=True)

            # pair loop: per pair two bf16 DVE ops, per group one batched Ln;
            # the first ngps TT-adds of each full group run on idle GPSIMD.
            gi = 0
            # migrated pairs go FIRST in the ACT stream: they depend only on
            # v0 (no DVE work), so the Ln pipeline starts while DVE still
            # builds the first group.
            for km in range(nmig):
                scol = Ms0 - nmig + km
                for bias_ap in (sig[:, scol:scol + 1],
                                pit[:, scol:scol + 1]):
                    dm = dpool.tile([128, N], bf16, tag="dump")
                    nc.scalar.activation(
                        out=dm[:], in_=vs[0][:], func=AF.Ln,
                        bias=bias_ap, accum_out=lacc[:, gi:gi + 1])
                    gi += 1
            for s in range(2):
                off = 0 if s == 0 else Ms0
                m0 = 0
                for gidx, gw in enumerate(gsizes[s]):
                    tg = tgpool.tile([128, GSIZE * N], bf16, tag="tg")
                    if bq:
                        # TS writes vs*sig+pi straight into the group buffer;
                        # one broadcast-Q tensor_tensor adds v^2 for the whole
                        # group (in1 stride-0 over the pair dim).
                        for k in range(gw):
                            scol = off + m0 + k
                            nc.vector.tensor_scalar(
                                out=tg[:, k * N:(k + 1) * N], in0=vs[s][:],
                                scalar1=sig[:, scol:scol + 1],
                                scalar2=pit[:, scol:scol + 1],
                                op0=OP.mult, op1=OP.add)
                        tgv = tg[:, :gw * N].rearrange("p (g n) -> p g n",
                                                       n=N)
                        qb = qs[s][:].unsqueeze(1).broadcast_to(
                            (128, gw, N))
                        nc.vector.tensor_tensor(out=tgv, in0=tgv, in1=qb,
                                                op=OP.add)
                    else:
                        for k in range(gw):
                            scol = off + m0 + k
                            t1 = t1pool.tile([128, N], bf16, tag="t1")
                            nc.vector.tensor_scalar(
                                out=t1[:], in0=vs[s][:],
                                scalar1=sig[:, scol:scol + 1],
                                scalar2=pit[:, scol:scol + 1],
                                op0=OP.mult, op1=OP.add)
                            eng = (nc.gpsimd if (gw >= 6 and k < ngps)
                                   else nc.vector)
                            eng.tensor_tensor(
                                out=tg[:, k * N:(k + 1) * N], in0=t1[:],
                                in1=qs[s][:], op=OP.add)
                    if inplace:
                        lnout = tg[:, :gw * N]
                    else:
                        dump = dpool.tile([128, GSIZE * N], bf16,
                                          tag="dumpg")
                        lnout = dump[:, :gw * N]
                    nc.scalar.activation(out=lnout, in_=tg[:, :gw * N],
                                         func=AF.Ln,
                                         accum_out=lacc[:, gi:gi + 1])
                    gi += 1
                    m0 += gw
                    if s == 0 and gidx == 0:
                        # slot Q1 behind the first chunk0 group
                        nc.vector.tensor_mul(qs[1][:], vs[1][:], vs[1][:])
                    if s == 1 and gidx == 0:
                        # chunk0 tail: by now its last Ln is done; overlaps
                        chunk_tail(0, 0)
            chunk_tail(1, NL[0])

            outs = cpool.tile([1, 2], f32)
            nc.vector.tensor_add(out=outs[0:1, 0:1], in0=part_sums[0][:],
                                 in1=part_sums[1][:])
            nc.vector.tensor_copy(out=outs[0:1, 1:2], in_=pq[:])
            nc.sync.dma_start(out=out_d[:], in_=outs[:])

    nc.finalize()
    return nc


# ---------------- v1 fallback (column-pairing, proven baseline) -----------


def _host_prep_v1(b, y):
    b = np.ascontiguousarray(np.asarray(b, dtype=np.float32))
    y = np.asarray(y, dtype=np.int64).ravel()
    n = b.shape[0]
    assert b.shape == (N, D) and y.shape == (N,), (b.shape, y.shape)

    b64 = b.astype(np.float64)
    labels, inv, counts = np.unique(y, return_inverse=True, return_counts=True)
    ncls = len(labels)
    n_row = counts[inv]

    order = np.argsort(-n_row, kind="stable")
    slot_rows = [order[: n // 2], order[n // 2:]]
    P0 = int(n_row[slot_rows[0]].max())
    P1 = int(n_row[slot_rows[1]].max())

    cls_idx = [np.nonzero(inv == c)[0] for c in range(ncls)]

    s_pos = 0.5 * (b64 * np.stack(
        [b64[ix].sum(axis=0) for ix in cls_idx])[inv]).sum(axis=1)
    s_all = 0.5 * (b64 @ b64.sum(axis=0))
    nc_r = n_row.astype(np.float64)
    npairs = nc_r * (n - nc_r)
    valid = (n_row >= 1) & (n_row < n)
    cnt = int(valid.sum())
    wvec_all = np.where(valid, 1.0 / np.maximum(npairs, 1.0) / max(cnt, 1), 0.0)

    pos_theta = [None] * n
    for ix in cls_idx:
        g = 0.5 * (b64[ix] @ b64[ix].T)
        for k, r in enumerate(ix):
            pos_theta[r] = g[k]

    bycls = np.argsort(inv, kind="stable")
    jperm = np.empty(n, dtype=np.int64)
    jperm[0::2] = bycls[: n // 2]
    jperm[1::2] = bycls[n // 2:]
    pair_sep = not np.any(inv[jperm[0::2]] == inv[jperm[1::2]])

    onehot = np.zeros((n, ncls), dtype=np.float32)
    onehot[np.arange(n), inv] = 1.0
    bth = np.concatenate([0.5 * b.T[:, jperm], onehot[jperm].T], axis=0)
    bth = np.ascontiguousarray(bth.astype(np.float32))

    sim_h = 0.5 * (b @ b.T)
    offmask = sim_h + 1000.0 * (y[:, None] == y[None, :])
    part = np.partition(offmask, 2, axis=1)[:, :2]
    v1 = np.exp(-part[:, 0].astype(np.float64))
    v2 = np.exp(-part[:, 1].astype(np.float64))
    cmax = np.exp(ALPHA - np.array([pt.min() for pt in pos_theta]))
    pair_ok = pair_sep and bool(
        max((v1 * v2).max(), (cmax * v1).max()) < 1.0e19)

    in_maps = []
    for core in range(NCORES):
        chunks = [slot_rows[0][core * 128:(core + 1) * 128],
                  slot_rows[1][core * 128:(core + 1) * 128]]
        rows = np.concatenate(chunks)
        brt = np.concatenate([b[rows].T, MASKC * onehot[rows].T], axis=0)
        brt = np.ascontiguousarray(brt.astype(np.float32))
        abias = np.full((128, P0 + P1), PAD_A, dtype=np.float32)
        tw = np.zeros((128, 4), dtype=np.float32)
        for s, (off, Ps, chunk) in enumerate(
                zip((0, P0), (P0, P1), chunks)):
            for p, r in enumerate(chunk):
                th = pos_theta[r]
                abias[p, off:off + th.size] = th - ALPHA
                ncr = nc_r[r]
                npad = Ps - ncr
                K = (Ps * s_all[r] - npad * s_pos[r]
                     + C_PAD * ncr * npad - ncr * ncr * ALPHA)
                tw[p, 2 * s] = -K
            tw[:, 2 * s + 1] = wvec_all[chunk]
        in_maps.append({"brt": brt, "bth": bth, "abias": abias, "tw": tw})
    return in_maps, P0, P1, ncls, pair_ok


def _build_bass_v1(P0, P1, ncls, dve_mod):
    import concourse.bacc as bacc
    import concourse.tile as tile
    from concourse import mybir

    f32 = mybir.dt.float32
    AF = mybir.ActivationFunctionType
    PT = P0 + P1
    KD = D + ncls

    nc = bacc.Bacc("TRN2", target_bir_lowering=False, debug=False,
                   num_devices=NCORES)
    brt_d = nc.dram_tensor("brt", [KD, 256], f32, kind="ExternalInput")
    bth_d = nc.dram_tensor("bth", [KD, N], f32, kind="ExternalInput")
    ab_d = nc.dram_tensor("abias", [128, PT], f32, kind="ExternalInput")
    tw_d = nc.dram_tensor("tw", [128, 4], f32, kind="ExternalInput")
    out_d = nc.dram_tensor("out", [1, 2], f32, kind="ExternalOutput")

    with tile.TileContext(nc) as tc:
        with (
            tc.tile_pool(name="const", bufs=1) as cpool,
            tc.tile_pool(name="scratch", bufs=3) as spool,
            tc.tile_pool(name="small", bufs=2) as mpool,
            tc.tile_pool(name="psum", bufs=2, space="PSUM") as ppool,
            tc.tile_pool(name="psum1", bufs=1, space="PSUM") as ppool1,
        ):
            brt = cpool.tile([KD, 256], f32)
            nc.sync.dma_start(out=brt[:], in_=brt_d[:])
            bth = cpool.tile([KD, N], f32)
            nc.sync.dma_start(out=bth[:], in_=bth_d[:])
            abias = cpool.tile([128, PT], f32)
            nc.sync.dma_start(out=abias[:], in_=ab_d[:])
            tw = cpool.tile([128, 4], f32)
            nc.sync.dma_start(out=tw[:], in_=tw_d[:])

            ones = cpool.tile([128, 1], f32)
            nc.vector.memset(ones[:], 1.0)

            uinv = cpool.tile([128, PT], f32)
            nc.scalar.activation(out=uinv[:], in_=abias[:], func=AF.Exp,
                                 scale=-1.0)
            vs = []
            for s in range(2):
                v = cpool.tile([128, N], f32, tag=f"v{s}")
                for q in range(N // 512):
                    pt = ppool.tile([128, 512], f32, tag="mm")
                    nc.tensor.matmul(pt[:], brt[:, s * 128:(s + 1) * 128],
                                     bth[:, q * 512:(q + 1) * 512],
                                     start=True, stop=True)
                    nc.scalar.activation(out=v[:, q * 512:(q + 1) * 512],
                                         in_=pt[:], func=AF.Exp, scale=-1.0)
                vs.append(v)

            bb = brt[:D, :]
            nb = mpool.tile([D, 256], f32, tag="nb")
            nc.vector.tensor_scalar_mul(nb[:], bb, -1.0)
            ab = mpool.tile([D, 256], f32, tag="ab")
            nc.vector.tensor_max(ab[:], bb, nb[:])
            nc.vector.tensor_scalar_add(ab[:], ab[:], -1.0)
            sq = mpool.tile([D, 256], f32, tag="sq")
            nc.vector.tensor_mul(sq[:], ab[:], ab[:])
            qcol = mpool.tile([D, 1], f32, tag="qcol")
            nc.vector.tensor_reduce(out=qcol[:], in_=sq[:],
                                    axis=mybir.AxisListType.X,
                                    op=mybir.AluOpType.add)
            pq = ppool1.tile([1, 1], f32, tag="pq")
            nc.tensor.matmul(pq[:], qcol[:], ones[:D, :], start=True, stop=True)

            part_sums = []
            for s, (off, Ps) in enumerate(((0, P0), (P0, P1))):
                lall = mpool.tile([128, Ps], f32, tag=f"lall{s}")
                for p in range(Ps):
                    ucol = uinv[:, off + p:off + p + 1]
                    if dve_mod and p % dve_mod != 0:
                        w = spool.tile([128, N], f32, tag="w")
                        nc.vector.tensor_scalar_add(w[:], vs[s][:], ucol)
                        wr = w[:].rearrange("q (a two) -> q a two", two=2)
                        m = spool.tile([128, N // 2], f32, tag="m")
                        nc.vector.tensor_mul(m[:], wr[:, :, 0], wr[:, :, 1])
                        mid = spool.tile([128, N // 2], f32, tag="mid")
                        nc.scalar.activation(out=mid[:], in_=m[:], func=AF.Ln,
                                             accum_out=lall[:, p:p + 1])
                        continue
                    big = spool.tile([128, N], f32, tag="big")
                    nc.scalar.activation(out=big[:], in_=vs[s][:], func=AF.Ln,
                                         bias=ucol,
                                         accum_out=lall[:, p:p + 1])
                la = mpool.tile([128, 1], f32, tag=f"la{s}")
                nc.vector.tensor_reduce(out=la[:], in_=lall[:],
                                        axis=mybir.AxisListType.X,
                                        op=mybir.AluOpType.add)
                r2 = mpool.tile([128, 1], f32, tag=f"r2{s}")
                nc.vector.tensor_sub(out=r2[:], in0=la[:],
                                     in1=tw[:, 2 * s:2 * s + 1])
                r3 = mpool.tile([128, 1], f32, tag=f"r3{s}")
                nc.vector.tensor_mul(out=r3[:], in0=r2[:],
                                     in1=tw[:, 2 * s + 1:2 * s + 2])
                pr = ppool1.tile([1, 1], f32, tag=f"pr{s}")
                nc.tensor.matmul(pr[:], r3[:], ones[:], start=True, stop=True)
                sb = mpool.tile([1, 1], f32, tag=f"sb{s}")
                nc.vector.tensor_copy(out=sb[:], in_=pr[:])
                part_sums.append(sb)

            outs = cpool.tile([1, 2], f32)
            nc.vector.tensor_add(out=outs[0:1, 0:1], in0=part_sums[0][:],
                                 in1=part_sums[1][:])
            nc.vector.tensor_copy(out=outs[0:1, 1:2], in_=pq[:])
            nc.sync.dma_start(out=out_d[:], in_=outs[:])

    nc.finalize()
    return nc


def kernel(b, y):
    global LAST_RESULTS
    from concourse.bass_utils import run_bass_kernel_spmd

    use_v2 = int(os.environ.get("BASS_DHN_V2", "1"))
    prep = _host_prep_v2(b, y) if use_v2 else None
    if prep is not None:
        in_maps, Ms0, Ms1, ncls, nmig = prep
        key = ("v2", Ms0, Ms1, ncls, nmig, GSIZE,
               os.environ.get("BASS_DHN_GPS", "0"),
               os.environ.get("BASS_DHN_BQ", "1"),
               os.environ.get("BASS_DHN_INPLACE", "1"))
        if key not in _CACHE:
            _CACHE[key] = _build_bass_v2(Ms0, Ms1, ncls, nmig)
        nc = _CACHE[key]
    else:
        in_maps, P0, P1, ncls, pair_ok = _host_prep_v1(b, y)
        dve_mod = 3 if pair_ok else 0
        key = ("v1", P0, P1, ncls, dve_mod)
        if key not in _CACHE:
            _CACHE[key] = _build_bass_v1(P0, P1, ncls, dve_mod)
        nc = _CACHE[key]

    trace = bool(int(os.environ.get("BASS_DHN_TRACE", "0")))
    res = run_bass_kernel_spmd(nc, in_maps, core_ids=list(range(NCORES)),
                               trace=trace)
    LAST_RESULTS = res

    loss1 = np.float64(0.0)
    loss2_sum = np.float64(0.0)
    for r in res.results:
        o = r["out"]
        loss1 += np.float64(o[0, 0])
        loss2_sum += np.float64(o[0, 1])
    loss2 = loss2_sum / (N * D)
    total = loss1 + LAMBDA * loss2
    return (np.float32(total), np.float32(loss1), np.float32(loss2))
